# revision 8
# baseline (speedup 1.0000x reference)
"""Trainium2 Bass kernel for the AGCA channel-gating module (gnn_message_passing).

Reference computation (per batch element b):
    m   = mean(x[b], over H,W)                  # (C,)
    y1  = w1 @ m                                # (HIDE,)
    s   = softmax(w2 * y1)                      # (HIDE,)
    y2  = y1 * s + A2.T @ y1                    # (HIDE,)
    y3  = relu(w3 * y2)                         # (HIDE,)
    g   = sigmoid(w4 @ y3)                      # (C,)
    out[b] = x[b] * g[:, None, None]

Memory-bound: 256 MB in + 256 MB out.  Strategy: pure data parallel over
batch (2 batch elements per NeuronCore).  Each 16 MB batch element is held
fully in SBUF so x is read exactly once: stream loads -> free-dim reduce
(DVE) trailing the loads -> tiny gate math (PE matmuls + ACT) -> per-channel
scale (ACT) -> stream stores.  Loads go on the sync HWDGE ring, stores on
the scalar HWDGE ring so they never queue behind each other; the weight pack
loads via the gpsimd (SWDGE) queue so the sync ring starts on x immediately.

All weights/constants are packed into one DRAM tensor ("wpack") loaded by a
single DMA so compute instructions have few distinct semaphore dependencies
(walrus's instruction encodings fit only one sync wait; Bacc legalizes the
rest, but fewer deps also schedule better).
"""

import numpy as np

import concourse.bass as bass
import concourse.mybir as mybir
import concourse.tile as tile
from concourse import bacc
from concourse.bass_utils import run_bass_kernel_spmd

B, C, H, W = 16, 256, 128, 128
HIDE = C // 2          # 128
NCORES = 8
BPC = B // NCORES      # batch elements per core = 2
HW = H * W             # 16384
P = 128                # SBUF partitions; C = 2 * P
NCHUNK = 8             # partial-sum chunks per channel half (reduce granularity)
LCHUNK = 4             # load/store chunks per channel half (2 MB DMA tiles)
F = HW // LCHUNK       # 4096 (2 MB tiles)
RED = HW // NCHUNK     # 2048 (reduce slice)
XBUFS = 11             # x-tile pool slots (22 MB of SBUF)
F32 = mybir.dt.float32
AX = mybir.AxisListType.X
AF = mybir.ActivationFunctionType

# wpack column layout (free dim), 128 partitions:
#   [0:256)    w1ts   lhsT chunks for y1 = w1 @ mean (mean divisor folded in)
#   [256:512)  w4t    w4.T
#   [512:640)  a2     A2
#   [640]      w2 broadcast   [641] w3 broadcast   [642] 1.0   [643] 0.0
#   [644:772)  row 0 holds 128 ones (lhsT for the partition-broadcast matmul)
WPACK_COLS = 772


def _build_nc():
    nc = bacc.Bacc(None, target_bir_lowering=False)

    x_ext = nc.declare_dram_parameter("x", [BPC, 2, P, HW], F32, isOutput=False)
    out_ext = nc.declare_dram_parameter("out", [BPC, 2, P, HW], F32, isOutput=True)
    wpack_ext = nc.declare_dram_parameter("wpack", [P, WPACK_COLS], F32,
                                          isOutput=False)

    with tile.TileContext(nc) as tc:
        with (
            tc.tile_pool(name="w", bufs=1) as wpool,
            tc.tile_pool(name="xp", bufs=XBUFS) as xpool,
            tc.tile_pool(name="sp", bufs=2) as spool,
            tc.tile_pool(name="pp", bufs=1, space=bass.MemorySpace.PSUM) as ppool,
        ):
            wpack = wpool.tile([P, WPACK_COLS], F32, tag="wpack")
            nc.gpsimd.dma_start(wpack[:], wpack_ext[:])

            # Warm-up ops consuming wpack on each compute engine: the engine
            # observes the wpack DMA semaphore here, so real instructions
            # below carry at most ONE sync wait each.
            warm = ppool.tile([1, 1], F32, tag="warm")
            nc.tensor.matmul(warm[:], wpack[0:1, 0:1], wpack[0:1, 0:1],
                             start=True, stop=True)
            wsc_a = spool.tile([P, 1], F32, tag="wsc_a")
            nc.scalar.copy(wsc_a[:], wpack[:, 643:644])
            wsc_v = spool.tile([P, 1], F32, tag="wsc_v")
            nc.vector.tensor_copy(wsc_v[:], wpack[:, 643:644])

            w1ts = wpack[:, 0:C]
            w4t = wpack[:, C:2 * C]
            a2 = wpack[:, 2 * C:2 * C + P]
            w2v = wpack[:, 640:641]
            w3v = wpack[:, 641:642]
            ones = wpack[:, 642:643]
            zeros = wpack[:, 643:644]
            onesr = wpack[0:1, 644:772]

            for b in range(BPC):
                acc = spool.tile([P, 2 * NCHUNK], F32, tag="acc")
                tiles = []
                for h in range(2):
                    for j in range(LCHUNK):
                        t = xpool.tile([P, F], F32, tag="x")
                        idx = h * LCHUNK + j
                        ldeng = nc.gpsimd if (b == 1 and idx % 2 == 1) else nc.sync
                        ldeng.dma_start(t[:], x_ext[b, h, :, j * F:(j + 1) * F])
                        k = h * NCHUNK + 2 * j
                        last = (idx == 2 * LCHUNK - 1)
                        nc.vector.reduce_sum(acc[:, k:k + 1], t[:, 0:RED], axis=AX)
                        if last:
                            # split the final reduce across ACT so the gate
                            # chain starts ~2us sooner
                            nc.scalar.activation(
                                t[:, RED:F], t[:, RED:F], AF.Copy,
                                accum_out=acc[:, k + 1:k + 2])
                        else:
                            nc.vector.reduce_sum(acc[:, k + 1:k + 2], t[:, RED:F],
                                                 axis=AX)
                        tiles.append((h, j, t))

                # ---- gate math (per-batch, tiny) ----
                # y1 = w1 @ mean: matmul straight on the per-chunk partial
                # sums (PSUM accumulates the channel halves), then one DVE
                # row-sum collapses the chunk axis PSUM->SBUF.
                y1p = ppool.tile([P, NCHUNK], F32, tag="y1p")
                nc.tensor.matmul(y1p[:], w1ts[:, 0:HIDE], acc[:, 0:NCHUNK],
                                 start=True, stop=False)
                nc.tensor.matmul(y1p[:], w1ts[:, HIDE:C], acc[:, NCHUNK:2 * NCHUNK],
                                 start=False, stop=True)
                y1 = spool.tile([P, 1], F32, tag="y1")
                nc.vector.reduce_sum(y1[:], y1p[:], axis=AX)

                # softmax(w2 * y1) over partitions (inputs are tiny -> no max
                # subtraction needed).  z = A2.T @ y1 and q = y1*e overlap
                # with the softmax-sum matmul chain.
                e = spool.tile([P, 1], F32, tag="e")
                nc.scalar.activation(e[:], y1[:], AF.Exp, bias=zeros, scale=w2v)
                zp = ppool.tile([P, 1], F32, tag="zp")
                nc.tensor.matmul(zp[:], a2[:], y1[:], start=True, stop=True)
                sump = ppool.tile([1, 1], F32, tag="sump")
                nc.tensor.matmul(sump[:], e[:], ones, start=True, stop=True)
                q = spool.tile([P, 1], F32, tag="q")
                nc.vector.tensor_mul(q[:], y1[:], e[:])
                r = spool.tile([1, 1], F32, tag="r")
                nc.vector.reciprocal(r[:], sump[:])
                rbp = ppool.tile([P, 1], F32, tag="rbp")
                nc.tensor.matmul(rbp[:], onesr[:], r[:], start=True, stop=True)

                # y2 = y1*softmax + A2.T@y1 = q/sum + z ; y3 = relu(w3*y2)
                y2 = spool.tile([P, 1], F32, tag="y2")
                nc.vector.tensor_mul(y2[:], q[:], rbp[:])
                nc.vector.tensor_add(y2[:], y2[:], zp[:])
                y3 = spool.tile([P, 1], F32, tag="y3")
                nc.scalar.activation(y3[:], y2[:], AF.Relu, bias=zeros, scale=w3v)

                # gate = sigmoid(w4 @ y3): two matmuls into one (128,2) PSUM
                # tile, one sigmoid over both columns.
                gp = ppool.tile([P, 2], F32, tag="gp")
                nc.tensor.matmul(gp[:, 0:1], w4t[:, 0:HIDE], y3[:],
                                 start=True, stop=True)
                nc.tensor.matmul(gp[:, 1:2], w4t[:, HIDE:C], y3[:],
                                 start=True, stop=True)
                gate = spool.tile([P, 2], F32, tag="gate")
                nc.scalar.activation(gate[:], gp[:], AF.Sigmoid,
                                     bias=zeros, scale=1.0)

                # ---- apply gate and store ----
                for idx, (h, j, t) in enumerate(tiles):
                    if b == 0:
                        nc.scalar.mul(t[:], t[:], gate[:, h:h + 1])
                        nc.scalar.dma_start(
                            out_ext[b, h, :, j * F:(j + 1) * F], t[:])
                    else:
                        nc.vector.tensor_scalar_mul(t[:], t[:], gate[:, h:h + 1])
                        eng = nc.sync if idx % 2 == 0 else nc.scalar
                        eng.dma_start(
                            out_ext[b, h, :, j * F:(j + 1) * F], t[:])

    nc.finalize()
    return nc


_NC_CACHE = {}


def _get_nc():
    if "nc" not in _NC_CACHE:
        _NC_CACHE["nc"] = _build_nc()
    return _NC_CACHE["nc"]


def _prep_in_maps(x, w1, w2, w3, w4, A2):
    x = np.ascontiguousarray(np.asarray(x, dtype=np.float32))
    w1 = np.asarray(w1, dtype=np.float32)
    w2 = float(np.asarray(w2))
    w3 = float(np.asarray(w3))
    w4 = np.asarray(w4, dtype=np.float32)
    A2 = np.asarray(A2, dtype=np.float32)

    wpack = np.zeros((P, WPACK_COLS), np.float32)
    # lhsT chunks for y1 = w1 @ (sums/HW): w1ts[k, h*HIDE+m] = w1[m, h*P+k]/HW
    w1t = (w1.T / float(HW)).astype(np.float32)          # (C, HIDE)
    wpack[:, 0:C] = w1t.reshape(2, P, HIDE).transpose(1, 0, 2).reshape(P, C)
    wpack[:, C:2 * C] = w4.T                             # (HIDE, C)
    wpack[:, 2 * C:2 * C + P] = A2
    wpack[:, 640] = w2
    wpack[:, 641] = w3
    wpack[:, 642] = 1.0
    wpack[:, 643] = 0.0
    wpack[0, 644:772] = 1.0

    in_maps = []
    for i in range(NCORES):
        shard = x[i * BPC:(i + 1) * BPC].reshape(BPC, 2, P, HW)
        in_maps.append({"x": shard, "wpack": wpack})
    return in_maps


def run(inputs, trace=False):
    """Run the kernel; returns (output, BassKernelResults)."""
    in_maps = _prep_in_maps(**inputs)
    nc = _get_nc()
    res = run_bass_kernel_spmd(nc, in_maps, core_ids=list(range(NCORES)),
                               trace=trace)
    out = np.empty((B, C, H, W), np.float32)
    for i in range(NCORES):
        out[i * BPC:(i + 1) * BPC] = np.asarray(
            res.results[i]["out"]).reshape(BPC, C, H, W)
    return out, res


def kernel(**inputs):
    out, _ = run(inputs, trace=False)
    return out


# revision 10
# speedup vs baseline: 1.0078x; 1.0078x over previous
"""Trainium2 Bass kernel for the AGCA channel-gating module (gnn_message_passing).

Reference computation (per batch element b):
    m   = mean(x[b], over H,W)                  # (C,)
    y1  = w1 @ m                                # (HIDE,)
    s   = softmax(w2 * y1)                      # (HIDE,)
    y2  = y1 * s + A2.T @ y1                    # (HIDE,)
    y3  = relu(w3 * y2)                         # (HIDE,)
    g   = sigmoid(w4 @ y3)                      # (C,)
    out[b] = x[b] * g[:, None, None]

Memory-bound: 256 MB in + 256 MB out.  Strategy: pure data parallel over
batch (2 batch elements per NeuronCore).  Each 16 MB batch element is held
fully in SBUF so x is read exactly once: stream loads -> free-dim reduce
(DVE) trailing the loads -> tiny gate math (PE matmuls + ACT) -> per-channel
scale (ACT) -> stream stores.  Loads go on the sync HWDGE ring, stores on
the scalar HWDGE ring so they never queue behind each other; the weight pack
loads via the gpsimd (SWDGE) queue so the sync ring starts on x immediately.

All weights/constants are packed into one DRAM tensor ("wpack") loaded by a
single DMA so compute instructions have few distinct semaphore dependencies
(walrus's instruction encodings fit only one sync wait; Bacc legalizes the
rest, but fewer deps also schedule better).
"""

import numpy as np

import concourse.bass as bass
import concourse.mybir as mybir
import concourse.tile as tile
from concourse import bacc
from concourse.bass_utils import run_bass_kernel_spmd

B, C, H, W = 16, 256, 128, 128
HIDE = C // 2          # 128
NCORES = 8
BPC = B // NCORES      # batch elements per core = 2
HW = H * W             # 16384
P = 128                # SBUF partitions; C = 2 * P
NCHUNK = 8             # partial-sum chunks per channel half (reduce granularity)
LCHUNK = 4             # load/store chunks per channel half (2 MB DMA tiles)
F = HW // LCHUNK       # 4096 (2 MB tiles)
RED = HW // NCHUNK     # 2048 (reduce slice)
XBUFS = 12             # x-tile pool slots (24 MB of SBUF)
F32 = mybir.dt.float32
AX = mybir.AxisListType.X
AF = mybir.ActivationFunctionType

# wpack column layout (free dim), 128 partitions:
#   [0:256)    w1ts   lhsT chunks for y1 = w1 @ mean (mean divisor folded in)
#   [256:512)  w4t    w4.T
#   [512:640)  a2     A2
#   [640]      w2 broadcast   [641] w3 broadcast   [642] 1.0   [643] 0.0
#   [644:772)  row 0 holds 128 ones (lhsT for the partition-broadcast matmul)
WPACK_COLS = 772


def _build_nc():
    nc = bacc.Bacc(None, target_bir_lowering=False)

    x_ext = nc.declare_dram_parameter("x", [BPC, 2, P, HW], F32, isOutput=False)
    out_ext = nc.declare_dram_parameter("out", [BPC, 2, P, HW], F32, isOutput=True)
    wpack_ext = nc.declare_dram_parameter("wpack", [P, WPACK_COLS], F32,
                                          isOutput=False)

    with tile.TileContext(nc) as tc:
        with (
            tc.tile_pool(name="w", bufs=1) as wpool,
            tc.tile_pool(name="xp", bufs=XBUFS) as xpool,
            tc.tile_pool(name="sp", bufs=2) as spool,
            tc.tile_pool(name="pp", bufs=1, space=bass.MemorySpace.PSUM) as ppool,
        ):
            wpack = wpool.tile([P, WPACK_COLS], F32, tag="wpack")
            nc.gpsimd.dma_start(wpack[:], wpack_ext[:])

            # Warm-up ops consuming wpack on each compute engine: the engine
            # observes the wpack DMA semaphore here, so real instructions
            # below carry at most ONE sync wait each.
            warm = ppool.tile([1, 1], F32, tag="warm")
            nc.tensor.matmul(warm[:], wpack[0:1, 0:1], wpack[0:1, 0:1],
                             start=True, stop=True)
            wsc_a = spool.tile([P, 1], F32, tag="wsc_a")
            nc.scalar.copy(wsc_a[:], wpack[:, 643:644])
            wsc_v = spool.tile([P, 1], F32, tag="wsc_v")
            nc.vector.tensor_copy(wsc_v[:], wpack[:, 643:644])

            w1ts = wpack[:, 0:C]
            w4t = wpack[:, C:2 * C]
            a2 = wpack[:, 2 * C:2 * C + P]
            w2v = wpack[:, 640:641]
            w3v = wpack[:, 641:642]
            ones = wpack[:, 642:643]
            zeros = wpack[:, 643:644]
            onesr = wpack[0:1, 644:772]

            for b in range(BPC):
                acc = spool.tile([P, 2 * NCHUNK], F32, tag="acc")
                tiles = []
                for h in range(2):
                    for j in range(LCHUNK):
                        t = xpool.tile([P, F], F32, tag="x")
                        idx = h * LCHUNK + j
                        nc.sync.dma_start(t[:], x_ext[b, h, :, j * F:(j + 1) * F])
                        k = h * NCHUNK + 2 * j
                        last = (idx == 2 * LCHUNK - 1)
                        nc.vector.reduce_sum(acc[:, k:k + 1], t[:, 0:RED], axis=AX)
                        if last:
                            # split the final reduce across ACT so the gate
                            # chain starts ~2us sooner
                            nc.scalar.activation(
                                t[:, RED:F], t[:, RED:F], AF.Copy,
                                accum_out=acc[:, k + 1:k + 2])
                        else:
                            nc.vector.reduce_sum(acc[:, k + 1:k + 2], t[:, RED:F],
                                                 axis=AX)
                        tiles.append((h, j, t))

                # ---- gate math (per-batch, tiny) ----
                # y1 = w1 @ mean: matmul straight on the per-chunk partial
                # sums (PSUM accumulates the channel halves), then one DVE
                # row-sum collapses the chunk axis PSUM->SBUF.
                y1p = ppool.tile([P, NCHUNK], F32, tag="y1p")
                nc.tensor.matmul(y1p[:], w1ts[:, 0:HIDE], acc[:, 0:NCHUNK],
                                 start=True, stop=False)
                nc.tensor.matmul(y1p[:], w1ts[:, HIDE:C], acc[:, NCHUNK:2 * NCHUNK],
                                 start=False, stop=True)
                y1 = spool.tile([P, 1], F32, tag="y1")
                nc.vector.reduce_sum(y1[:], y1p[:], axis=AX)

                # softmax(w2 * y1) over partitions (inputs are tiny -> no max
                # subtraction needed).  z = A2.T @ y1 and q = y1*e overlap
                # with the softmax-sum matmul chain.
                e = spool.tile([P, 1], F32, tag="e")
                nc.scalar.activation(e[:], y1[:], AF.Exp, bias=zeros, scale=w2v)
                zp = ppool.tile([P, 1], F32, tag="zp")
                nc.tensor.matmul(zp[:], a2[:], y1[:], start=True, stop=True)
                sump = ppool.tile([1, 1], F32, tag="sump")
                nc.tensor.matmul(sump[:], e[:], ones, start=True, stop=True)
                q = spool.tile([P, 1], F32, tag="q")
                nc.vector.tensor_mul(q[:], y1[:], e[:])
                r = spool.tile([1, 1], F32, tag="r")
                nc.vector.reciprocal(r[:], sump[:])
                rbp = ppool.tile([P, 1], F32, tag="rbp")
                nc.tensor.matmul(rbp[:], onesr[:], r[:], start=True, stop=True)

                # y2 = y1*softmax + A2.T@y1 = q/sum + z ; y3 = relu(w3*y2)
                y2 = spool.tile([P, 1], F32, tag="y2")
                nc.vector.tensor_mul(y2[:], q[:], rbp[:])
                nc.vector.tensor_add(y2[:], y2[:], zp[:])
                y3 = spool.tile([P, 1], F32, tag="y3")
                nc.scalar.activation(y3[:], y2[:], AF.Relu, bias=zeros, scale=w3v)

                # gate = sigmoid(w4 @ y3): two matmuls into one (128,2) PSUM
                # tile, one sigmoid over both columns.
                gp = ppool.tile([P, 2], F32, tag="gp")
                nc.tensor.matmul(gp[:, 0:1], w4t[:, 0:HIDE], y3[:],
                                 start=True, stop=True)
                nc.tensor.matmul(gp[:, 1:2], w4t[:, HIDE:C], y3[:],
                                 start=True, stop=True)
                gate = spool.tile([P, 2], F32, tag="gate")
                nc.scalar.activation(gate[:], gp[:], AF.Sigmoid,
                                     bias=zeros, scale=1.0)

                # ---- apply gate and store ----
                for idx, (h, j, t) in enumerate(tiles):
                    if b == 0:
                        nc.scalar.mul(t[:], t[:], gate[:, h:h + 1])
                        nc.scalar.dma_start(
                            out_ext[b, h, :, j * F:(j + 1) * F], t[:])
                    else:
                        nc.vector.tensor_scalar_mul(t[:], t[:], gate[:, h:h + 1])
                        eng = nc.sync if idx % 2 == 0 else nc.scalar
                        eng.dma_start(
                            out_ext[b, h, :, j * F:(j + 1) * F], t[:])

    nc.finalize()
    return nc


_NC_CACHE = {}


def _get_nc():
    if "nc" not in _NC_CACHE:
        _NC_CACHE["nc"] = _build_nc()
    return _NC_CACHE["nc"]


def _prep_in_maps(x, w1, w2, w3, w4, A2):
    x = np.ascontiguousarray(np.asarray(x, dtype=np.float32))
    w1 = np.asarray(w1, dtype=np.float32)
    w2 = float(np.asarray(w2))
    w3 = float(np.asarray(w3))
    w4 = np.asarray(w4, dtype=np.float32)
    A2 = np.asarray(A2, dtype=np.float32)

    wpack = np.zeros((P, WPACK_COLS), np.float32)
    # lhsT chunks for y1 = w1 @ (sums/HW): w1ts[k, h*HIDE+m] = w1[m, h*P+k]/HW
    w1t = (w1.T / float(HW)).astype(np.float32)          # (C, HIDE)
    wpack[:, 0:C] = w1t.reshape(2, P, HIDE).transpose(1, 0, 2).reshape(P, C)
    wpack[:, C:2 * C] = w4.T                             # (HIDE, C)
    wpack[:, 2 * C:2 * C + P] = A2
    wpack[:, 640] = w2
    wpack[:, 641] = w3
    wpack[:, 642] = 1.0
    wpack[:, 643] = 0.0
    wpack[0, 644:772] = 1.0

    in_maps = []
    for i in range(NCORES):
        shard = x[i * BPC:(i + 1) * BPC].reshape(BPC, 2, P, HW)
        in_maps.append({"x": shard, "wpack": wpack})
    return in_maps


def run(inputs, trace=False):
    """Run the kernel; returns (output, BassKernelResults)."""
    in_maps = _prep_in_maps(**inputs)
    nc = _get_nc()
    res = run_bass_kernel_spmd(nc, in_maps, core_ids=list(range(NCORES)),
                               trace=trace)
    out = np.empty((B, C, H, W), np.float32)
    for i in range(NCORES):
        out[i * BPC:(i + 1) * BPC] = np.asarray(
            res.results[i]["out"]).reshape(BPC, C, H, W)
    return out, res


def kernel(**inputs):
    out, _ = run(inputs, trace=False)
    return out


# revision 11
# speedup vs baseline: 1.0098x; 1.0020x over previous
"""Trainium2 Bass kernel for the AGCA channel-gating module (gnn_message_passing).

Reference computation (per batch element b):
    m   = mean(x[b], over H,W)                  # (C,)
    y1  = w1 @ m                                # (HIDE,)
    s   = softmax(w2 * y1)                      # (HIDE,)
    y2  = y1 * s + A2.T @ y1                    # (HIDE,)
    y3  = relu(w3 * y2)                         # (HIDE,)
    g   = sigmoid(w4 @ y3)                      # (C,)
    out[b] = x[b] * g[:, None, None]

Memory-bound: 256 MB in + 256 MB out.  Strategy: pure data parallel over
batch (2 batch elements per NeuronCore).  Each 16 MB batch element is held
fully in SBUF so x is read exactly once: stream loads -> free-dim reduce
(DVE) trailing the loads -> tiny gate math (PE matmuls + ACT) -> per-channel
scale (ACT) -> stream stores.  Loads go on the sync HWDGE ring, stores on
the scalar HWDGE ring so they never queue behind each other; the weight pack
loads via the gpsimd (SWDGE) queue so the sync ring starts on x immediately.

All weights/constants are packed into one DRAM tensor ("wpack") loaded by a
single DMA so compute instructions have few distinct semaphore dependencies
(walrus's instruction encodings fit only one sync wait; Bacc legalizes the
rest, but fewer deps also schedule better).
"""

import numpy as np

import concourse.bass as bass
import concourse.mybir as mybir
import concourse.tile as tile
from concourse import bacc
from concourse.bass_utils import run_bass_kernel_spmd

B, C, H, W = 16, 256, 128, 128
HIDE = C // 2          # 128
NCORES = 8
BPC = B // NCORES      # batch elements per core = 2
HW = H * W             # 16384
P = 128                # SBUF partitions; C = 2 * P
NCHUNK = 8             # partial-sum chunks per channel half (reduce granularity)
LCHUNK = 4             # load/store chunks per channel half (2 MB DMA tiles)
F = HW // LCHUNK       # 4096 (2 MB tiles)
RED = HW // NCHUNK     # 2048 (reduce slice)
XBUFS = 12             # x-tile pool slots (24 MB of SBUF)
F32 = mybir.dt.float32
AX = mybir.AxisListType.X
AF = mybir.ActivationFunctionType

# wpack column layout (free dim), 128 partitions:
#   [0:256)    w1ts   lhsT chunks for y1 = w1 @ mean (mean divisor folded in)
#   [256:512)  w4t    w4.T
#   [512:640)  a2     A2
#   [640]      w2 broadcast   [641] w3 broadcast   [642] 1.0   [643] 0.0
#   [644:772)  row 0 holds 128 ones (lhsT for the partition-broadcast matmul)
WPACK_COLS = 772


def _build_nc():
    nc = bacc.Bacc(None, target_bir_lowering=False)

    x_ext = nc.declare_dram_parameter("x", [BPC, 2, P, HW], F32, isOutput=False)
    out_ext = nc.declare_dram_parameter("out", [BPC, 2, P, HW], F32, isOutput=True)
    wpack_ext = nc.declare_dram_parameter("wpack", [P, WPACK_COLS], F32,
                                          isOutput=False)

    with tile.TileContext(nc) as tc:
        with (
            tc.tile_pool(name="w", bufs=1) as wpool,
            tc.tile_pool(name="xp", bufs=XBUFS) as xpool,
            tc.tile_pool(name="sp", bufs=2) as spool,
            tc.tile_pool(name="pp", bufs=1, space=bass.MemorySpace.PSUM) as ppool,
        ):
            wpack = wpool.tile([P, WPACK_COLS], F32, tag="wpack")
            nc.gpsimd.dma_start(wpack[:], wpack_ext[:])

            # Warm-up ops consuming wpack on each compute engine: the engine
            # observes the wpack DMA semaphore here, so real instructions
            # below carry at most ONE sync wait each.
            warm = ppool.tile([1, 1], F32, tag="warm")
            nc.tensor.matmul(warm[:], wpack[0:1, 0:1], wpack[0:1, 0:1],
                             start=True, stop=True)
            wsc_a = spool.tile([P, 1], F32, tag="wsc_a")
            nc.scalar.copy(wsc_a[:], wpack[:, 643:644])
            wsc_v = spool.tile([P, 1], F32, tag="wsc_v")
            nc.vector.tensor_copy(wsc_v[:], wpack[:, 643:644])

            w1ts = wpack[:, 0:C]
            w4t = wpack[:, C:2 * C]
            a2 = wpack[:, 2 * C:2 * C + P]
            w2v = wpack[:, 640:641]
            w3v = wpack[:, 641:642]
            ones = wpack[:, 642:643]
            zeros = wpack[:, 643:644]
            onesr = wpack[0:1, 644:772]

            for b in range(BPC):
                acc = spool.tile([P, 2 * NCHUNK], F32, tag="acc")
                tiles = []
                for h in range(2):
                    for j in range(LCHUNK):
                        t = xpool.tile([P, F], F32, tag="x")
                        idx = h * LCHUNK + j
                        nc.sync.dma_start(t[:], x_ext[b, h, :, j * F:(j + 1) * F])
                        k = h * NCHUNK + 2 * j
                        nc.vector.reduce_sum(acc[:, k:k + 1], t[:, 0:RED], axis=AX)
                        nc.vector.reduce_sum(acc[:, k + 1:k + 2], t[:, RED:F],
                                             axis=AX)
                        tiles.append((h, j, t))

                # ---- gate math (per-batch, tiny) ----
                # y1 = w1 @ mean: matmul straight on the per-chunk partial
                # sums (PSUM accumulates the channel halves), then one DVE
                # row-sum collapses the chunk axis PSUM->SBUF.
                y1p = ppool.tile([P, NCHUNK], F32, tag="y1p")
                nc.tensor.matmul(y1p[:], w1ts[:, 0:HIDE], acc[:, 0:NCHUNK],
                                 start=True, stop=False)
                nc.tensor.matmul(y1p[:], w1ts[:, HIDE:C], acc[:, NCHUNK:2 * NCHUNK],
                                 start=False, stop=True)
                y1 = spool.tile([P, 1], F32, tag="y1")
                nc.vector.reduce_sum(y1[:], y1p[:], axis=AX)

                # softmax(w2 * y1) over partitions (inputs are tiny -> no max
                # subtraction needed).  z = A2.T @ y1 and q = y1*e overlap
                # with the softmax-sum matmul chain.
                e = spool.tile([P, 1], F32, tag="e")
                nc.scalar.activation(e[:], y1[:], AF.Exp, bias=zeros, scale=w2v)
                zp = ppool.tile([P, 1], F32, tag="zp")
                nc.tensor.matmul(zp[:], a2[:], y1[:], start=True, stop=True)
                sump = ppool.tile([1, 1], F32, tag="sump")
                nc.tensor.matmul(sump[:], e[:], ones, start=True, stop=True)
                q = spool.tile([P, 1], F32, tag="q")
                nc.vector.tensor_mul(q[:], y1[:], e[:])
                r = spool.tile([1, 1], F32, tag="r")
                nc.vector.reciprocal(r[:], sump[:])
                rbp = ppool.tile([P, 1], F32, tag="rbp")
                nc.tensor.matmul(rbp[:], onesr[:], r[:], start=True, stop=True)

                # y2 = y1*softmax + A2.T@y1 = q/sum + z ; y3 = relu(w3*y2)
                y2 = spool.tile([P, 1], F32, tag="y2")
                nc.vector.tensor_mul(y2[:], q[:], rbp[:])
                nc.vector.tensor_add(y2[:], y2[:], zp[:])
                y3 = spool.tile([P, 1], F32, tag="y3")
                nc.scalar.activation(y3[:], y2[:], AF.Relu, bias=zeros, scale=w3v)

                # gate = sigmoid(w4 @ y3): two matmuls into one (128,2) PSUM
                # tile, one sigmoid over both columns.
                gp = ppool.tile([P, 2], F32, tag="gp")
                nc.tensor.matmul(gp[:, 0:1], w4t[:, 0:HIDE], y3[:],
                                 start=True, stop=True)
                nc.tensor.matmul(gp[:, 1:2], w4t[:, HIDE:C], y3[:],
                                 start=True, stop=True)
                gate = spool.tile([P, 2], F32, tag="gate")
                nc.scalar.activation(gate[:], gp[:], AF.Sigmoid,
                                     bias=zeros, scale=1.0)

                # ---- apply gate and store ----
                for idx, (h, j, t) in enumerate(tiles):
                    if b == 0:
                        nc.scalar.mul(t[:], t[:], gate[:, h:h + 1])
                        nc.scalar.dma_start(
                            out_ext[b, h, :, j * F:(j + 1) * F], t[:])
                    else:
                        nc.vector.tensor_scalar_mul(t[:], t[:], gate[:, h:h + 1])
                        eng = nc.sync if idx % 2 == 0 else nc.scalar
                        eng.dma_start(
                            out_ext[b, h, :, j * F:(j + 1) * F], t[:])

    nc.finalize()
    return nc


_NC_CACHE = {}


def _get_nc():
    if "nc" not in _NC_CACHE:
        _NC_CACHE["nc"] = _build_nc()
    return _NC_CACHE["nc"]


def _prep_in_maps(x, w1, w2, w3, w4, A2):
    x = np.ascontiguousarray(np.asarray(x, dtype=np.float32))
    w1 = np.asarray(w1, dtype=np.float32)
    w2 = float(np.asarray(w2))
    w3 = float(np.asarray(w3))
    w4 = np.asarray(w4, dtype=np.float32)
    A2 = np.asarray(A2, dtype=np.float32)

    wpack = np.zeros((P, WPACK_COLS), np.float32)
    # lhsT chunks for y1 = w1 @ (sums/HW): w1ts[k, h*HIDE+m] = w1[m, h*P+k]/HW
    w1t = (w1.T / float(HW)).astype(np.float32)          # (C, HIDE)
    wpack[:, 0:C] = w1t.reshape(2, P, HIDE).transpose(1, 0, 2).reshape(P, C)
    wpack[:, C:2 * C] = w4.T                             # (HIDE, C)
    wpack[:, 2 * C:2 * C + P] = A2
    wpack[:, 640] = w2
    wpack[:, 641] = w3
    wpack[:, 642] = 1.0
    wpack[:, 643] = 0.0
    wpack[0, 644:772] = 1.0

    in_maps = []
    for i in range(NCORES):
        shard = x[i * BPC:(i + 1) * BPC].reshape(BPC, 2, P, HW)
        in_maps.append({"x": shard, "wpack": wpack})
    return in_maps


def run(inputs, trace=False):
    """Run the kernel; returns (output, BassKernelResults)."""
    in_maps = _prep_in_maps(**inputs)
    nc = _get_nc()
    res = run_bass_kernel_spmd(nc, in_maps, core_ids=list(range(NCORES)),
                               trace=trace)
    out = np.empty((B, C, H, W), np.float32)
    for i in range(NCORES):
        out[i * BPC:(i + 1) * BPC] = np.asarray(
            res.results[i]["out"]).reshape(BPC, C, H, W)
    return out, res


def kernel(**inputs):
    out, _ = run(inputs, trace=False)
    return out
